# revision 3
# baseline (speedup 1.0000x reference)
"""Trainium2 Bass kernel for nn_CrossAttention (CroCo RoPE2D cross-attention).

Sharding: 8 cores = batch(4) x head-group(2, 8 heads each).
Per core: QKV projections (fp32r matmuls), RoPE2D via host-precomputed
cos/sin tables (channel-major layout, stream_shuffle for rotate-half),
attention with scoresT [nk, nq] layout (row-packed K=64 head pairs),
exp on ACT with 1/8 scale folded in, PV + softmax sums via ones-matmul
(col-packed), normalization with fast reciprocal, output projection.

Output per core: partial out [2048, 1024] for its head group; host sums
the two group partials per batch and adds the bias.
"""

import numpy as np

import concourse.bass as bass
import concourse.mybir as mybir
import concourse.tile as tile
from concourse import bacc
from concourse.bass import ds, ts
from concourse.bass_utils import run_bass_kernel_spmd

B, NQ, NK, C = 4, 2048, 2048, 1024
H, DH = 16, 64
G = 2                      # head groups (tensor-parallel)
CS = C // G                # 512 channels per core
HPC = H // G               # 8 heads per core
NCORES = 8
P = 128
KIO = C // P               # 8 contraction subtiles for projections
NKT = NK // P              # 16 nk tiles
NCHUNK = 512               # nq chunk for attention stages
NCHUNKS = NQ // NCHUNK     # 4

f32 = mybir.dt.float32
f32r = mybir.dt.float32r
bf16 = mybir.dt.bfloat16
EXP = mybir.ActivationFunctionType.Exp
SWAP16 = [(i + 16) % 32 for i in range(32)]
SCALE = DH ** -0.5


def _emit_kernel(nc: bass.Bass):
    xq = nc.dram_tensor("xq", [C, NQ], f32, kind="ExternalInput").ap()
    xk = nc.dram_tensor("xk", [C, NK], f32, kind="ExternalInput").ap()
    xv = nc.dram_tensor("xv", [C, NK], f32, kind="ExternalInput").ap()
    wq = nc.dram_tensor("wq", [C, CS], f32, kind="ExternalInput").ap()
    wk = nc.dram_tensor("wk", [C, CS], f32, kind="ExternalInput").ap()
    wv = nc.dram_tensor("wv", [C, CS], f32, kind="ExternalInput").ap()
    wo = nc.dram_tensor("wo", [CS, C], f32, kind="ExternalInput").ap()
    cq = nc.dram_tensor("cq", [P, NQ], f32, kind="ExternalInput").ap()
    sq = nc.dram_tensor("sq", [P, NQ], f32, kind="ExternalInput").ap()
    ck = nc.dram_tensor("ck", [P, NK], f32, kind="ExternalInput").ap()
    sk = nc.dram_tensor("sk", [P, NK], f32, kind="ExternalInput").ap()
    out = nc.dram_tensor("out", [NQ, C], f32, kind="ExternalOutput").ap()

    xq_r = xq.rearrange("(io p) n -> p io n", p=P)
    xk_r = xk.rearrange("(io p) n -> p io n", p=P)
    xv_r = xv.rearrange("(io p) n -> p io n", p=P)
    wq_r = wq.rearrange("(io p) c -> p io c", p=P).bitcast(f32r)
    wk_r = wk.rearrange("(io p) c -> p io c", p=P).bitcast(f32r)
    wv_r = wv.rearrange("(io p) c -> p io c", p=P).bitcast(f32r)
    wo_r = wo.rearrange("(co p) o -> p co o", p=P).bitcast(f32r)

    with tile.TileContext(nc) as tc:
        with tc.tile_pool(name="persist", bufs=1) as persist:
            # Persistent SBUF state
            qT = [persist.tile([P, NQ], f32r, name=f"qT{i}", tag=f"qT{i}")
                  for i in range(4)]
            kT = [persist.tile([P, NK], f32r, name=f"kT{i}", tag=f"kT{i}")
                  for i in range(4)]
            v_sb = persist.tile([P, NKT, CS], bf16, name="v_sb", tag="v_sb")
            ones = persist.tile([P, 64], bf16, name="ones", tag="ones")
            woT = persist.tile([P, CS // P, C], f32r, name="woT", tag="woT")

            nc.vector.memset(ones[:], 1.0)
            nc.sync.dma_start(woT[:], wo_r)

            # ---------------- Phase A: projections + RoPE ----------------
            with tc.tile_pool(name="stream", bufs=2) as stream, \
                 tc.tile_pool(name="ropep", bufs=2) as ropep, \
                 tc.tile_pool(name="tabs", bufs=1) as tabs, \
                 tc.tile_pool(name="psA", bufs=2, space="PSUM") as psA:

                # v projection first: PV needs it earliest is false, but
                # freeing its x-stream early simplifies SBUF. Emit q/k for
                # ct=0 first so attention can start early, then v, then rest.
                def proj_qk(x_r, w_r, cos_dram, sin_dram, dstT, wtag, cts):
                    cos_sb = tabs.tile([P, NQ], f32, name=f"cos_{wtag}",
                                       tag="cos", bufs=1)
                    sin_sb = tabs.tile([P, NQ], f32, name=f"sin_{wtag}",
                                       tag="sin", bufs=1)
                    nc.sync.dma_start(cos_sb[:], cos_dram)
                    nc.sync.dma_start(sin_sb[:], sin_dram)
                    w_sb = stream.tile([P, KIO, CS], f32r, name=f"w_{wtag}",
                                       tag="w", bufs=1)
                    nc.sync.dma_start(w_sb[:], w_r)
                    for ch in range(NCHUNKS):
                        x_sb = stream.tile([P, KIO, NCHUNK], f32r,
                                           name=f"x_{wtag}{ch}", tag="x",
                                           bufs=2)
                        nc.sync.dma_start(
                            x_sb[:], x_r[:, :, ds(ch * NCHUNK, NCHUNK)].bitcast(f32r)
                        )
                        for ct in cts:
                            pst = psA.tile([P, NCHUNK], f32, name=f"pj{wtag}{ch}{ct}",
                                           tag="proj", bufs=2)
                            for kio in range(KIO):
                                nc.tensor.matmul(
                                    pst[:],
                                    w_sb[:, kio, ds(ct * P, P)],
                                    x_sb[:, kio, :],
                                    start=(kio == 0), stop=(kio == KIO - 1),
                                )
                            qs = ropep.tile([P, NCHUNK], f32, name=f"qs{wtag}{ch}{ct}",
                                            tag="qs", bufs=2)
                            nc.vector.stream_shuffle(qs[:], pst[:], SWAP16)
                            t1 = ropep.tile([P, NCHUNK], f32, name=f"t1{wtag}{ch}{ct}",
                                            tag="t1", bufs=2)
                            nc.vector.tensor_mul(
                                out=t1[:], in0=pst[:],
                                in1=cos_sb[:, ds(ch * NCHUNK, NCHUNK)],
                            )
                            t2 = ropep.tile([P, NCHUNK], f32, name=f"t2{wtag}{ch}{ct}",
                                            tag="t2", bufs=2)
                            nc.vector.tensor_mul(
                                out=t2[:], in0=qs[:],
                                in1=sin_sb[:, ds(ch * NCHUNK, NCHUNK)],
                            )
                            nc.vector.tensor_add(
                                out=dstT[ct][:, ds(ch * NCHUNK, NCHUNK)],
                                in0=t1[:], in1=t2[:],
                            )

                proj_qk(xq_r, wq_r, cq, sq, qT, "q", range(4))
                proj_qk(xk_r, wk_r, ck, sk, kT, "k", range(4))

                # v projection: natural [nk, c] layout
                wv_sb = stream.tile([P, KIO, CS], f32r, name="wv_sb", tag="w",
                                    bufs=1)
                nc.sync.dma_start(wv_sb[:], wv_r)
                for t in range(NKT):
                    xv_sb = stream.tile([P, KIO, P], f32r, name=f"xv{t}",
                                        tag="xv", bufs=2)
                    nc.sync.dma_start(
                        xv_sb[:], xv_r[:, :, ds(t * P, P)].bitcast(f32r)
                    )
                    psv = psA.tile([P, CS], f32, name=f"psv{t}", tag="proj",
                                   bufs=2)
                    for kio in range(KIO):
                        nc.tensor.matmul(
                            psv[:], xv_sb[:, kio, :], wv_sb[:, kio, :],
                            start=(kio == 0), stop=(kio == KIO - 1),
                        )
                    nc.vector.tensor_copy(v_sb[:, t, :], psv[:])

            # ---------------- Phase B: attention + output proj ----------------
            with tc.tile_pool(name="sbB", bufs=2) as sbB, \
                 tc.tile_pool(name="psB", bufs=1, space="PSUM") as psB:

                for ch in range(NCHUNKS):
                    nq_sl = ds(ch * NCHUNK, NCHUNK)
                    xn = [sbB.tile([P, NCHUNK], f32r, name=f"xn{ch}_{ct}",
                                   tag=f"xn{ct}", bufs=2) for ct in range(4)]
                    for pair in range(4):
                        # scoresT: [nk-tile, nq] for head A (rows 0:64 of
                        # qT/kT tile `pair`) and head B (rows 64:128),
                        # row-packed concurrent matmuls.
                        pv = psB.tile([P, NCHUNK], f32, name=f"pv{ch}{pair}",
                                      tag="pv", bufs=1)
                        sums = psB.tile([P, NCHUNK], f32, name=f"sm{ch}{pair}",
                                        tag="sums", bufs=1)
                        for t in range(NKT):
                            qk = psB.tile([P, 2 * NCHUNK], f32,
                                          name=f"qk{ch}{pair}{t}", tag="qk",
                                          bufs=2)
                            nc.tensor.matmul(
                                qk[:, 0:NCHUNK],
                                kT[pair][0:64, ds(t * P, P)],
                                qT[pair][0:64, nq_sl],
                                start=True, stop=True,
                            )
                            nc.tensor.matmul(
                                qk[:, NCHUNK:2 * NCHUNK],
                                kT[pair][64:128, ds(t * P, P)],
                                qT[pair][64:128, nq_sl],
                                start=True, stop=True,
                            )
                            e = sbB.tile([P, 2 * NCHUNK], bf16, name=f"e{ch}{pair}{t}",
                                         tag="e", bufs=3)
                            nc.scalar.activation(e[:], qk[:], EXP, scale=SCALE)
                            # PV + sums, col-packed pairs
                            hA = 2 * pair
                            hB = 2 * pair + 1
                            nc.tensor.matmul(
                                pv[0:64, :], v_sb[:, t, ds(hA * DH, DH)],
                                e[:, 0:NCHUNK],
                                start=(t == 0), stop=(t == NKT - 1),
                            )
                            nc.tensor.matmul(
                                pv[64:128, :], v_sb[:, t, ds(hB * DH, DH)],
                                e[:, NCHUNK:2 * NCHUNK],
                                start=(t == 0), stop=(t == NKT - 1),
                            )
                            nc.tensor.matmul(
                                sums[0:64, :], ones[:], e[:, 0:NCHUNK],
                                start=(t == 0), stop=(t == NKT - 1),
                            )
                            nc.tensor.matmul(
                                sums[64:128, :], ones[:], e[:, NCHUNK:2 * NCHUNK],
                                start=(t == 0), stop=(t == NKT - 1),
                            )
                        recips = sbB.tile([P, NCHUNK], f32, name=f"rc{ch}{pair}",
                                          tag="recips", bufs=2)
                        nc.vector.reciprocal_approx_fast(out=recips[:], in_=sums[:])
                        nc.vector.tensor_mul(
                            out=xn[pair][:], in0=pv[:], in1=recips[:],
                        )

                    # Output projection for this nq chunk:
                    # out[nq, o] += xn[ct].T @ woT[ct]
                    for ns in range(NCHUNK // P):
                        o_sb = sbB.tile([P, C], f32, name=f"o{ch}{ns}",
                                        tag="osb", bufs=2)
                        for oc in range(2):
                            wops = psB.tile([P, 512], f32, name=f"wops{ch}{ns}{oc}",
                                            tag="sums", bufs=1)
                            for ct in range(4):
                                nc.tensor.matmul(
                                    wops[:],
                                    xn[ct][:, ds(ns * P, P)],
                                    woT[:, ct, ds(oc * 512, 512)],
                                    start=(ct == 0), stop=(ct == 3),
                                )
                            nc.vector.tensor_copy(o_sb[:, ds(oc * 512, 512)], wops[:])
                        nc.sync.dma_start(
                            out[ds(ch * NCHUNK + ns * P, P), :], o_sb[:]
                        )
    return nc


_NC_CACHE = None


def _get_nc():
    global _NC_CACHE
    if _NC_CACHE is None:
        nc = bacc.Bacc("TRN2", target_bir_lowering=False, debug=False)
        _emit_kernel(nc)
        nc.compile()
        _NC_CACHE = nc
    return _NC_CACHE


def _rope_tables(pos):
    """pos: [N, 2] int -> cos/sin tables [128, N] f32, channel-major.

    Channel c in [0,64): j = c % 32 selects rotate-half slot, jj = j % 16
    the frequency; c < 32 uses pos[:, 0] (y), else pos[:, 1] (x). Sign of
    sin folded in: negative for j < 16. Rows 64:128 repeat 0:64 (2 heads
    per 128-partition tile).
    """
    invf = 1.0 / (100.0 ** (np.arange(0, 32, 2, dtype=np.float32) / 32.0))
    ay = (pos[:, 0].astype(np.float32)[None, :] * invf[:, None])  # [16, N]
    ax = (pos[:, 1].astype(np.float32)[None, :] * invf[:, None])
    cos = np.concatenate([np.cos(ay), np.cos(ay), np.cos(ax), np.cos(ax)], 0)
    sin = np.concatenate([-np.sin(ay), np.sin(ay), -np.sin(ax), np.sin(ax)], 0)
    cos = np.concatenate([cos, cos], 0).astype(np.float32)  # [128, N]
    sin = np.concatenate([sin, sin], 0).astype(np.float32)
    return np.ascontiguousarray(cos), np.ascontiguousarray(sin)


def _build_in_maps(query, key, value, qpos, kpos, Wq, Wk, Wv, Wo):
    xqT = [np.ascontiguousarray(query[b].T) for b in range(B)]
    xkT = [np.ascontiguousarray(key[b].T) for b in range(B)]
    xvT = [np.ascontiguousarray(value[b].T) for b in range(B)]
    tabs_q = [_rope_tables(np.asarray(qpos[b])) for b in range(B)]
    tabs_k = [_rope_tables(np.asarray(kpos[b])) for b in range(B)]
    wqT = [np.ascontiguousarray(Wq[g * CS:(g + 1) * CS, :].T) for g in range(G)]
    wkT = [np.ascontiguousarray(Wk[g * CS:(g + 1) * CS, :].T) for g in range(G)]
    wvT = [np.ascontiguousarray(Wv[g * CS:(g + 1) * CS, :].T) for g in range(G)]
    woT = [np.ascontiguousarray(Wo[:, g * CS:(g + 1) * CS].T) for g in range(G)]

    in_maps = []
    for core in range(NCORES):
        b, g = core // G, core % G
        in_maps.append({
            "xq": xqT[b], "xk": xkT[b], "xv": xvT[b],
            "wq": wqT[g], "wk": wkT[g], "wv": wvT[g], "wo": woT[g],
            "cq": tabs_q[b][0], "sq": tabs_q[b][1],
            "ck": tabs_k[b][0], "sk": tabs_k[b][1],
        })
    return in_maps


def run_sharded(inputs, trace=False):
    """Run the SPMD kernel; returns (results, BassKernelResults)."""
    nc = _get_nc()
    in_maps = _build_in_maps(
        inputs["query"], inputs["key"], inputs["value"],
        inputs["qpos"], inputs["kpos"],
        inputs["Wq"], inputs["Wk"], inputs["Wv"], inputs["Wo"],
    )
    res = run_bass_kernel_spmd(nc, in_maps, list(range(NCORES)), trace=trace)
    return res


def _gather(results, bo):
    out = np.empty((B, NQ, C), np.float32)
    for b in range(B):
        out[b] = results[b * G]["out"] + results[b * G + 1]["out"] + bo
    return out


def kernel(query, key, value, qpos, kpos, Wq, Wk, Wv, Wo, bo):
    inputs = {
        "query": np.asarray(query, np.float32),
        "key": np.asarray(key, np.float32),
        "value": np.asarray(value, np.float32),
        "qpos": np.asarray(qpos),
        "kpos": np.asarray(kpos),
        "Wq": np.asarray(Wq, np.float32),
        "Wk": np.asarray(Wk, np.float32),
        "Wv": np.asarray(Wv, np.float32),
        "Wo": np.asarray(Wo, np.float32),
    }
    res = run_sharded(inputs, trace=False)
    return _gather(res.results, np.asarray(bo, np.float32))


# revision 4
# speedup vs baseline: 115.8746x; 115.8746x over previous
"""Trainium2 Bass kernel for nn_CrossAttention (CroCo RoPE2D cross-attention).

Sharding: 8 cores = batch(4) x head-group(2, 8 heads each).
Per core: QKV projections (fp32r matmuls), RoPE2D via host-precomputed
cos/sin tables (channel-major layout, stream_shuffle for rotate-half),
attention with scoresT [nk, nq] layout (row-packed K=64 head pairs),
exp on ACT with 1/8 scale folded in, PV + softmax sums via ones-matmul
(col-packed), normalization with fast reciprocal, output projection.

Output per core: partial out [2048, 1024] for its head group; host sums
the two group partials per batch and adds the bias.
"""

import numpy as np

import concourse.bass as bass
import concourse.mybir as mybir
import concourse.tile as tile
from concourse import bacc
from concourse.bass import ds, ts
from concourse.bass_utils import run_bass_kernel_spmd

B, NQ, NK, C = 4, 2048, 2048, 1024
H, DH = 16, 64
G = 2                      # head groups (tensor-parallel)
CS = C // G                # 512 channels per core
HPC = H // G               # 8 heads per core
NCORES = 8
P = 128
KIO = C // P               # 8 contraction subtiles for projections
NKT = NK // P              # 16 nk tiles
NCHUNK = 512               # nq chunk for attention stages
NCHUNKS = NQ // NCHUNK     # 4

f32 = mybir.dt.float32
f32r = mybir.dt.float32r
bf16 = mybir.dt.bfloat16
EXP = mybir.ActivationFunctionType.Exp
SWAP16 = [(i + 16) % 32 for i in range(32)]
SCALE = DH ** -0.5


def _emit_kernel(nc: bass.Bass, repeats: int = 1):
    xq = nc.dram_tensor("xq", [C, NQ], f32, kind="ExternalInput").ap()
    xk = nc.dram_tensor("xk", [C, NK], f32, kind="ExternalInput").ap()
    xv = nc.dram_tensor("xv", [C, NK], f32, kind="ExternalInput").ap()
    wq = nc.dram_tensor("wq", [C, CS], f32, kind="ExternalInput").ap()
    wk = nc.dram_tensor("wk", [C, CS], f32, kind="ExternalInput").ap()
    wv = nc.dram_tensor("wv", [C, CS], f32, kind="ExternalInput").ap()
    wo = nc.dram_tensor("wo", [CS, C], f32, kind="ExternalInput").ap()
    cq = nc.dram_tensor("cq", [P, NQ], f32, kind="ExternalInput").ap()
    sq = nc.dram_tensor("sq", [P, NQ], f32, kind="ExternalInput").ap()
    ck = nc.dram_tensor("ck", [P, NK], f32, kind="ExternalInput").ap()
    sk = nc.dram_tensor("sk", [P, NK], f32, kind="ExternalInput").ap()
    out = nc.dram_tensor("out", [NQ, C], f32, kind="ExternalOutput").ap()

    xq_r = xq.rearrange("(io p) n -> p io n", p=P)
    xk_r = xk.rearrange("(io p) n -> p io n", p=P)
    xv_r = xv.rearrange("(io p) n -> p io n", p=P)
    wq_r = wq.rearrange("(io p) c -> p io c", p=P).bitcast(f32r)
    wk_r = wk.rearrange("(io p) c -> p io c", p=P).bitcast(f32r)
    wv_r = wv.rearrange("(io p) c -> p io c", p=P).bitcast(f32r)
    wo_r = wo.rearrange("(co p) o -> p co o", p=P).bitcast(f32r)

    with tile.TileContext(nc) as tc:
      for _rep in range(repeats):
        px = f"r{_rep}_" if repeats > 1 else ""
        with tc.tile_pool(name=px + "persist", bufs=1) as persist:
            # Persistent SBUF state
            qT = [persist.tile([P, NQ], f32r, name=f"{px}qT{i}", tag=f"{px}qT{i}")
                  for i in range(4)]
            kT = [persist.tile([P, NK], f32r, name=f"{px}kT{i}", tag=f"{px}kT{i}")
                  for i in range(4)]
            v_sb = persist.tile([P, NKT, CS], bf16, name=px + "v_sb", tag=px + "v_sb")
            ones = persist.tile([P, 64], bf16, name=px + "ones", tag=px + "ones")
            woT = persist.tile([P, CS // P, C], f32r, name=px + "woT", tag=px + "woT")

            nc.vector.memset(ones[:], 1.0)
            nc.sync.dma_start(woT[:], wo_r)

            # ---------------- Phase A: projections + RoPE ----------------
            with tc.tile_pool(name=px + "stream", bufs=2) as stream, \
                 tc.tile_pool(name=px + "ropep", bufs=2) as ropep, \
                 tc.tile_pool(name=px + "tabs", bufs=1) as tabs, \
                 tc.tile_pool(name=px + "psA", bufs=2, space="PSUM") as psA:

                # v projection first: PV needs it earliest is false, but
                # freeing its x-stream early simplifies SBUF. Emit q/k for
                # ct=0 first so attention can start early, then v, then rest.
                def proj_qk(x_r, w_r, cos_dram, sin_dram, dstT, wtag, cts):
                    cos_sb = tabs.tile([P, NQ], f32, name=f"{px}cos_{wtag}",
                                       tag=px + "cos", bufs=1)
                    sin_sb = tabs.tile([P, NQ], f32, name=f"{px}sin_{wtag}",
                                       tag=px + "sin", bufs=1)
                    nc.sync.dma_start(cos_sb[:], cos_dram)
                    nc.sync.dma_start(sin_sb[:], sin_dram)
                    w_sb = stream.tile([P, KIO, CS], f32r, name=f"{px}w_{wtag}",
                                       tag=px + "w", bufs=1)
                    nc.sync.dma_start(w_sb[:], w_r)
                    for ch in range(NCHUNKS):
                        x_sb = stream.tile([P, KIO, NCHUNK], f32r,
                                           name=f"{px}x_{wtag}{ch}", tag=px + "x",
                                           bufs=2)
                        nc.sync.dma_start(
                            x_sb[:], x_r[:, :, ds(ch * NCHUNK, NCHUNK)].bitcast(f32r)
                        )
                        for ct in cts:
                            pst = psA.tile([P, NCHUNK], f32, name=f"{px}pj{wtag}{ch}{ct}",
                                           tag=px + "proj", bufs=2)
                            for kio in range(KIO):
                                nc.tensor.matmul(
                                    pst[:],
                                    w_sb[:, kio, ds(ct * P, P)],
                                    x_sb[:, kio, :],
                                    start=(kio == 0), stop=(kio == KIO - 1),
                                )
                            qs = ropep.tile([P, NCHUNK], f32, name=f"{px}qs{wtag}{ch}{ct}",
                                            tag=px + "qs", bufs=2)
                            nc.vector.stream_shuffle(qs[:], pst[:], SWAP16)
                            t1 = ropep.tile([P, NCHUNK], f32, name=f"{px}t1{wtag}{ch}{ct}",
                                            tag=px + "t1", bufs=2)
                            nc.vector.tensor_mul(
                                out=t1[:], in0=pst[:],
                                in1=cos_sb[:, ds(ch * NCHUNK, NCHUNK)],
                            )
                            t2 = ropep.tile([P, NCHUNK], f32, name=f"{px}t2{wtag}{ch}{ct}",
                                            tag=px + "t2", bufs=2)
                            nc.vector.tensor_mul(
                                out=t2[:], in0=qs[:],
                                in1=sin_sb[:, ds(ch * NCHUNK, NCHUNK)],
                            )
                            nc.vector.tensor_add(
                                out=dstT[ct][:, ds(ch * NCHUNK, NCHUNK)],
                                in0=t1[:], in1=t2[:],
                            )

                proj_qk(xq_r, wq_r, cq, sq, qT, "q", range(4))
                proj_qk(xk_r, wk_r, ck, sk, kT, "k", range(4))

                # v projection: natural [nk, c] layout
                wv_sb = stream.tile([P, KIO, CS], f32r, name=px + "wv_sb", tag=px + "w",
                                    bufs=1)
                nc.sync.dma_start(wv_sb[:], wv_r)
                for t in range(NKT):
                    xv_sb = stream.tile([P, KIO, P], f32r, name=f"{px}xv{t}",
                                        tag=px + "xv", bufs=2)
                    nc.sync.dma_start(
                        xv_sb[:], xv_r[:, :, ds(t * P, P)].bitcast(f32r)
                    )
                    psv = psA.tile([P, CS], f32, name=f"{px}psv{t}", tag=px + "proj",
                                   bufs=2)
                    for kio in range(KIO):
                        nc.tensor.matmul(
                            psv[:], xv_sb[:, kio, :], wv_sb[:, kio, :],
                            start=(kio == 0), stop=(kio == KIO - 1),
                        )
                    nc.vector.tensor_copy(v_sb[:, t, :], psv[:])

            # ---------------- Phase B: attention + output proj ----------------
            with tc.tile_pool(name=px + "sbB", bufs=2) as sbB, \
                 tc.tile_pool(name=px + "psB", bufs=1, space="PSUM") as psB:

                for ch in range(NCHUNKS):
                    nq_sl = ds(ch * NCHUNK, NCHUNK)
                    xn = [sbB.tile([P, NCHUNK], f32r, name=f"{px}xn{ch}_{ct}",
                                   tag=f"{px}xn{ct}", bufs=2) for ct in range(4)]
                    for pair in range(4):
                        # scoresT: [nk-tile, nq] for head A (rows 0:64 of
                        # qT/kT tile `pair`) and head B (rows 64:128),
                        # row-packed concurrent matmuls.
                        pv = psB.tile([P, NCHUNK], f32, name=f"{px}pv{ch}{pair}",
                                      tag=px + "pv", bufs=1)
                        sums = psB.tile([P, NCHUNK], f32, name=f"{px}sm{ch}{pair}",
                                        tag=px + "sums", bufs=1)
                        for t in range(NKT):
                            qk = psB.tile([P, 2 * NCHUNK], f32,
                                          name=f"{px}qk{ch}{pair}{t}", tag=px + "qk",
                                          bufs=2)
                            nc.tensor.matmul(
                                qk[:, 0:NCHUNK],
                                kT[pair][0:64, ds(t * P, P)],
                                qT[pair][0:64, nq_sl],
                                start=True, stop=True,
                            )
                            nc.tensor.matmul(
                                qk[:, NCHUNK:2 * NCHUNK],
                                kT[pair][64:128, ds(t * P, P)],
                                qT[pair][64:128, nq_sl],
                                start=True, stop=True,
                            )
                            e = sbB.tile([P, 2 * NCHUNK], bf16, name=f"{px}e{ch}{pair}{t}",
                                         tag=px + "e", bufs=3)
                            nc.scalar.activation(e[:], qk[:], EXP, scale=SCALE)
                            # PV + sums, col-packed pairs
                            hA = 2 * pair
                            hB = 2 * pair + 1
                            nc.tensor.matmul(
                                pv[0:64, :], v_sb[:, t, ds(hA * DH, DH)],
                                e[:, 0:NCHUNK],
                                start=(t == 0), stop=(t == NKT - 1),
                            )
                            nc.tensor.matmul(
                                pv[64:128, :], v_sb[:, t, ds(hB * DH, DH)],
                                e[:, NCHUNK:2 * NCHUNK],
                                start=(t == 0), stop=(t == NKT - 1),
                            )
                            nc.tensor.matmul(
                                sums[0:64, :], ones[:], e[:, 0:NCHUNK],
                                start=(t == 0), stop=(t == NKT - 1),
                            )
                            nc.tensor.matmul(
                                sums[64:128, :], ones[:], e[:, NCHUNK:2 * NCHUNK],
                                start=(t == 0), stop=(t == NKT - 1),
                            )
                        recips = sbB.tile([P, NCHUNK], f32, name=f"{px}rc{ch}{pair}",
                                          tag=px + "recips", bufs=2)
                        nc.vector.reciprocal_approx_fast(out=recips[:], in_=sums[:])
                        nc.vector.tensor_mul(
                            out=xn[pair][:], in0=pv[:], in1=recips[:],
                        )

                    # Output projection for this nq chunk:
                    # out[nq, o] += xn[ct].T @ woT[ct]
                    for ns in range(NCHUNK // P):
                        o_sb = sbB.tile([P, C], f32, name=f"{px}o{ch}{ns}",
                                        tag=px + "osb", bufs=2)
                        for oc in range(2):
                            wops = psB.tile([P, 512], f32, name=f"{px}wops{ch}{ns}{oc}",
                                            tag=px + "sums", bufs=1)
                            for ct in range(4):
                                nc.tensor.matmul(
                                    wops[:],
                                    xn[ct][:, ds(ns * P, P)],
                                    woT[:, ct, ds(oc * 512, 512)],
                                    start=(ct == 0), stop=(ct == 3),
                                )
                            nc.vector.tensor_copy(o_sb[:, ds(oc * 512, 512)], wops[:])
                        nc.sync.dma_start(
                            out[ds(ch * NCHUNK + ns * P, P), :], o_sb[:]
                        )
    return nc


_NC_CACHE = {}


def _get_nc(repeats: int = 1):
    if repeats not in _NC_CACHE:
        nc = bacc.Bacc("TRN2", target_bir_lowering=False, debug=False)
        _emit_kernel(nc, repeats)
        nc.compile()
        _NC_CACHE[repeats] = nc
    return _NC_CACHE[repeats]


def _rope_tables(pos):
    """pos: [N, 2] int -> cos/sin tables [128, N] f32, channel-major.

    Channel c in [0,64): j = c % 32 selects rotate-half slot, jj = j % 16
    the frequency; c < 32 uses pos[:, 0] (y), else pos[:, 1] (x). Sign of
    sin folded in: negative for j < 16. Rows 64:128 repeat 0:64 (2 heads
    per 128-partition tile).
    """
    invf = 1.0 / (100.0 ** (np.arange(0, 32, 2, dtype=np.float32) / 32.0))
    ay = (pos[:, 0].astype(np.float32)[None, :] * invf[:, None])  # [16, N]
    ax = (pos[:, 1].astype(np.float32)[None, :] * invf[:, None])
    cos = np.concatenate([np.cos(ay), np.cos(ay), np.cos(ax), np.cos(ax)], 0)
    sin = np.concatenate([-np.sin(ay), np.sin(ay), -np.sin(ax), np.sin(ax)], 0)
    cos = np.concatenate([cos, cos], 0).astype(np.float32)  # [128, N]
    sin = np.concatenate([sin, sin], 0).astype(np.float32)
    return np.ascontiguousarray(cos), np.ascontiguousarray(sin)


def _build_in_maps(query, key, value, qpos, kpos, Wq, Wk, Wv, Wo):
    xqT = [np.ascontiguousarray(query[b].T) for b in range(B)]
    xkT = [np.ascontiguousarray(key[b].T) for b in range(B)]
    xvT = [np.ascontiguousarray(value[b].T) for b in range(B)]
    tabs_q = [_rope_tables(np.asarray(qpos[b])) for b in range(B)]
    tabs_k = [_rope_tables(np.asarray(kpos[b])) for b in range(B)]
    wqT = [np.ascontiguousarray(Wq[g * CS:(g + 1) * CS, :].T) for g in range(G)]
    wkT = [np.ascontiguousarray(Wk[g * CS:(g + 1) * CS, :].T) for g in range(G)]
    wvT = [np.ascontiguousarray(Wv[g * CS:(g + 1) * CS, :].T) for g in range(G)]
    woT = [np.ascontiguousarray(Wo[:, g * CS:(g + 1) * CS].T) for g in range(G)]

    in_maps = []
    for core in range(NCORES):
        b, g = core // G, core % G
        in_maps.append({
            "xq": xqT[b], "xk": xkT[b], "xv": xvT[b],
            "wq": wqT[g], "wk": wkT[g], "wv": wvT[g], "wo": woT[g],
            "cq": tabs_q[b][0], "sq": tabs_q[b][1],
            "ck": tabs_k[b][0], "sk": tabs_k[b][1],
        })
    return in_maps


def run_sharded(inputs, trace=False):
    """Run the SPMD kernel; returns (results, BassKernelResults)."""
    nc = _get_nc()
    in_maps = _build_in_maps(
        inputs["query"], inputs["key"], inputs["value"],
        inputs["qpos"], inputs["kpos"],
        inputs["Wq"], inputs["Wk"], inputs["Wv"], inputs["Wo"],
    )
    res = run_bass_kernel_spmd(nc, in_maps, list(range(NCORES)), trace=trace)
    return res


def _gather(results, bo):
    out = np.empty((B, NQ, C), np.float32)
    for b in range(B):
        out[b] = results[b * G]["out"] + results[b * G + 1]["out"] + bo
    return out


def kernel(query, key, value, qpos, kpos, Wq, Wk, Wv, Wo, bo):
    inputs = {
        "query": np.asarray(query, np.float32),
        "key": np.asarray(key, np.float32),
        "value": np.asarray(value, np.float32),
        "qpos": np.asarray(qpos),
        "kpos": np.asarray(kpos),
        "Wq": np.asarray(Wq, np.float32),
        "Wk": np.asarray(Wk, np.float32),
        "Wv": np.asarray(Wv, np.float32),
        "Wo": np.asarray(Wo, np.float32),
    }
    res = run_sharded(inputs, trace=False)
    return _gather(res.results, np.asarray(bo, np.float32))


# revision 5
# speedup vs baseline: 272.9073x; 2.3552x over previous
"""Trainium2 Bass kernel for nn_CrossAttention (CroCo RoPE2D cross-attention).

Sharding: 8 cores = batch(4) x head-group(2, 8 heads each).
Per core: QKV projections (fp32r matmuls), RoPE2D via host-precomputed
cos/sin tables (channel-major layout, stream_shuffle for rotate-half),
attention with scoresT [nk, nq] layout (row-packed K=64 head pairs),
exp on ACT with 1/8 scale folded in, PV + softmax sums via ones-matmul
(col-packed), normalization with fast reciprocal, output projection.

Output per core: partial out [2048, 1024] for its head group; host sums
the two group partials per batch and adds the bias.
"""

import numpy as np

import concourse.bass as bass
import concourse.mybir as mybir
import concourse.tile as tile
from concourse import bacc
from concourse.bass import ds, ts
from concourse.bass_utils import run_bass_kernel_spmd

B, NQ, NK, C = 4, 2048, 2048, 1024
H, DH = 16, 64
G = 2                      # head groups (tensor-parallel)
CS = C // G                # 512 channels per core
HPC = H // G               # 8 heads per core
NCORES = 8
P = 128
KIO = C // P               # 8 contraction subtiles for projections
NKT = NK // P              # 16 nk tiles
NCHUNK = 512               # nq chunk for attention stages
NCHUNKS = NQ // NCHUNK     # 4

f32 = mybir.dt.float32
f32r = mybir.dt.float32r
bf16 = mybir.dt.bfloat16
EXP = mybir.ActivationFunctionType.Exp
SWAP16 = [(i + 16) % 32 for i in range(32)]
SCALE = DH ** -0.5


def _emit_kernel(nc: bass.Bass, repeats: int = 1):
    xq = nc.dram_tensor("xq", [NCHUNKS, P, KIO, NCHUNK], f32, kind="ExternalInput").ap()
    xk = nc.dram_tensor("xk", [NCHUNKS, P, KIO, NCHUNK], f32, kind="ExternalInput").ap()
    xv = nc.dram_tensor("xv", [NKT, P, KIO, P], f32, kind="ExternalInput").ap()
    wq = nc.dram_tensor("wq", [P, KIO, CS], f32, kind="ExternalInput").ap()
    wk = nc.dram_tensor("wk", [P, KIO, CS], f32, kind="ExternalInput").ap()
    wv = nc.dram_tensor("wv", [P, KIO, CS], f32, kind="ExternalInput").ap()
    wo = nc.dram_tensor("wo", [P, CS // P, C], f32, kind="ExternalInput").ap()
    cq = nc.dram_tensor("cq", [P, NQ], f32, kind="ExternalInput").ap()
    sq = nc.dram_tensor("sq", [P, NQ], f32, kind="ExternalInput").ap()
    ck = nc.dram_tensor("ck", [P, NK], f32, kind="ExternalInput").ap()
    sk = nc.dram_tensor("sk", [P, NK], f32, kind="ExternalInput").ap()
    out = nc.dram_tensor("out", [NQ, C], f32, kind="ExternalOutput").ap()

    xq_r, xk_r, xv_r = xq, xk, xv
    wq_r = wq.bitcast(f32r)
    wk_r = wk.bitcast(f32r)
    wv_r = wv.bitcast(f32r)
    wo_r = wo.bitcast(f32r)

    with tile.TileContext(nc) as tc:
      for _rep in range(repeats):
        px = f"r{_rep}_" if repeats > 1 else ""
        with tc.tile_pool(name=px + "persist", bufs=1) as persist:
            # Persistent SBUF state
            qT = [persist.tile([P, NQ], f32r, name=f"{px}qT{i}", tag=f"{px}qT{i}")
                  for i in range(4)]
            kT = [persist.tile([P, NK], f32r, name=f"{px}kT{i}", tag=f"{px}kT{i}")
                  for i in range(4)]
            v_sb = persist.tile([P, NKT, CS], bf16, name=px + "v_sb", tag=px + "v_sb")
            ones = persist.tile([P, 64], bf16, name=px + "ones", tag=px + "ones")
            woT = persist.tile([P, CS // P, C], f32r, name=px + "woT", tag=px + "woT")

            nc.vector.memset(ones[:], 1.0)
            nc.sync.dma_start(woT[:], wo_r)

            # ---------------- Phase A: projections + RoPE ----------------
            with tc.tile_pool(name=px + "stream", bufs=2) as stream, \
                 tc.tile_pool(name=px + "ropep", bufs=2) as ropep, \
                 tc.tile_pool(name=px + "tabs", bufs=1) as tabs, \
                 tc.tile_pool(name=px + "psA", bufs=2, space="PSUM") as psA:

                # v projection first: PV needs it earliest is false, but
                # freeing its x-stream early simplifies SBUF. Emit q/k for
                # ct=0 first so attention can start early, then v, then rest.
                def proj_qk(x_r, w_r, cos_dram, sin_dram, dstT, wtag, cts):
                    cos_sb = tabs.tile([P, NQ], f32, name=f"{px}cos_{wtag}",
                                       tag=px + "cos", bufs=1)
                    sin_sb = tabs.tile([P, NQ], f32, name=f"{px}sin_{wtag}",
                                       tag=px + "sin", bufs=1)
                    nc.sync.dma_start(cos_sb[:], cos_dram)
                    nc.sync.dma_start(sin_sb[:], sin_dram)
                    w_sb = stream.tile([P, KIO, CS], f32r, name=f"{px}w_{wtag}",
                                       tag=px + "w", bufs=1)
                    nc.sync.dma_start(w_sb[:], w_r)
                    for ch in range(NCHUNKS):
                        x_sb = stream.tile([P, KIO, NCHUNK], f32r,
                                           name=f"{px}x_{wtag}{ch}", tag=px + "x",
                                           bufs=2)
                        nc.sync.dma_start(x_sb[:], x_r[ch].bitcast(f32r))
                        for ct in cts:
                            pst = psA.tile([P, NCHUNK], f32, name=f"{px}pj{wtag}{ch}{ct}",
                                           tag=px + "proj", bufs=2)
                            for kio in range(KIO):
                                nc.tensor.matmul(
                                    pst[:],
                                    w_sb[:, kio, ds(ct * P, P)],
                                    x_sb[:, kio, :],
                                    start=(kio == 0), stop=(kio == KIO - 1),
                                )
                            qs = ropep.tile([P, NCHUNK], f32, name=f"{px}qs{wtag}{ch}{ct}",
                                            tag=px + "qs", bufs=2)
                            nc.vector.stream_shuffle(qs[:], pst[:], SWAP16)
                            t1 = ropep.tile([P, NCHUNK], f32, name=f"{px}t1{wtag}{ch}{ct}",
                                            tag=px + "t1", bufs=2)
                            nc.vector.tensor_mul(
                                out=t1[:], in0=pst[:],
                                in1=cos_sb[:, ds(ch * NCHUNK, NCHUNK)],
                            )
                            t2 = ropep.tile([P, NCHUNK], f32, name=f"{px}t2{wtag}{ch}{ct}",
                                            tag=px + "t2", bufs=2)
                            nc.vector.tensor_mul(
                                out=t2[:], in0=qs[:],
                                in1=sin_sb[:, ds(ch * NCHUNK, NCHUNK)],
                            )
                            nc.vector.tensor_add(
                                out=dstT[ct][:, ds(ch * NCHUNK, NCHUNK)],
                                in0=t1[:], in1=t2[:],
                            )

                proj_qk(xq_r, wq_r, cq, sq, qT, "q", range(4))
                proj_qk(xk_r, wk_r, ck, sk, kT, "k", range(4))

                # v projection: natural [nk, c] layout
                wv_sb = stream.tile([P, KIO, CS], f32r, name=px + "wv_sb", tag=px + "w",
                                    bufs=1)
                nc.sync.dma_start(wv_sb[:], wv_r)
                for t in range(NKT):
                    xv_sb = stream.tile([P, KIO, P], f32r, name=f"{px}xv{t}",
                                        tag=px + "xv", bufs=4)
                    nc.sync.dma_start(xv_sb[:], xv_r[t].bitcast(f32r))
                    psv = psA.tile([P, CS], f32, name=f"{px}psv{t}", tag=px + "proj",
                                   bufs=2)
                    for kio in range(KIO):
                        nc.tensor.matmul(
                            psv[:], xv_sb[:, kio, :], wv_sb[:, kio, :],
                            start=(kio == 0), stop=(kio == KIO - 1),
                        )
                    nc.vector.tensor_copy(v_sb[:, t, :], psv[:])

            # ---------------- Phase B: attention + output proj ----------------
            with tc.tile_pool(name=px + "sbB", bufs=2) as sbB, \
                 tc.tile_pool(name=px + "psB", bufs=1, space="PSUM") as psB:

                for ch in range(NCHUNKS):
                    nq_sl = ds(ch * NCHUNK, NCHUNK)
                    xn = [sbB.tile([P, NCHUNK], f32r, name=f"{px}xn{ch}_{ct}",
                                   tag=f"{px}xn{ct}", bufs=2) for ct in range(4)]
                    for pair in range(4):
                        # scoresT: [nk-tile, nq] for head A (rows 0:64 of
                        # qT/kT tile `pair`) and head B (rows 64:128),
                        # row-packed concurrent matmuls.
                        pv = psB.tile([P, NCHUNK], f32, name=f"{px}pv{ch}{pair}",
                                      tag=px + "pv", bufs=1)
                        sums = psB.tile([P, NCHUNK], f32, name=f"{px}sm{ch}{pair}",
                                        tag=px + "sums", bufs=1)
                        for t in range(NKT):
                            qk = psB.tile([P, 2 * NCHUNK], f32,
                                          name=f"{px}qk{ch}{pair}{t}", tag=px + "qk",
                                          bufs=2)
                            nc.tensor.matmul(
                                qk[:, 0:NCHUNK],
                                kT[pair][0:64, ds(t * P, P)],
                                qT[pair][0:64, nq_sl],
                                start=True, stop=True,
                            )
                            nc.tensor.matmul(
                                qk[:, NCHUNK:2 * NCHUNK],
                                kT[pair][64:128, ds(t * P, P)],
                                qT[pair][64:128, nq_sl],
                                start=True, stop=True,
                            )
                            e = sbB.tile([P, 2 * NCHUNK], bf16, name=f"{px}e{ch}{pair}{t}",
                                         tag=px + "e", bufs=6)
                            nc.scalar.activation(e[:], qk[:], EXP, scale=SCALE)
                            # PV + sums, col-packed pairs
                            hA = 2 * pair
                            hB = 2 * pair + 1
                            nc.tensor.matmul(
                                pv[0:64, :], v_sb[:, t, ds(hA * DH, DH)],
                                e[:, 0:NCHUNK],
                                start=(t == 0), stop=(t == NKT - 1),
                            )
                            nc.tensor.matmul(
                                pv[64:128, :], v_sb[:, t, ds(hB * DH, DH)],
                                e[:, NCHUNK:2 * NCHUNK],
                                start=(t == 0), stop=(t == NKT - 1),
                            )
                            nc.tensor.matmul(
                                sums[0:64, :], ones[:], e[:, 0:NCHUNK],
                                start=(t == 0), stop=(t == NKT - 1),
                            )
                            nc.tensor.matmul(
                                sums[64:128, :], ones[:], e[:, NCHUNK:2 * NCHUNK],
                                start=(t == 0), stop=(t == NKT - 1),
                            )
                        recips = sbB.tile([P, NCHUNK], f32, name=f"{px}rc{ch}{pair}",
                                          tag=px + "recips", bufs=2)
                        nc.vector.reciprocal_approx_fast(out=recips[:], in_=sums[:])
                        nc.vector.tensor_mul(
                            out=xn[pair][:], in0=pv[:], in1=recips[:],
                        )

                    # Output projection for this nq chunk:
                    # out[nq, o] += xn[ct].T @ woT[ct]
                    for ns in range(NCHUNK // P):
                        o_sb = sbB.tile([P, C], f32, name=f"{px}o{ch}{ns}",
                                        tag=px + "osb", bufs=2)
                        for oc in range(2):
                            wops = psB.tile([P, 512], f32, name=f"{px}wops{ch}{ns}{oc}",
                                            tag=px + "sums", bufs=1)
                            for ct in range(4):
                                nc.tensor.matmul(
                                    wops[:],
                                    xn[ct][:, ds(ns * P, P)],
                                    woT[:, ct, ds(oc * 512, 512)],
                                    start=(ct == 0), stop=(ct == 3),
                                )
                            nc.vector.tensor_copy(o_sb[:, ds(oc * 512, 512)], wops[:])
                        nc.sync.dma_start(
                            out[ds(ch * NCHUNK + ns * P, P), :], o_sb[:]
                        )
    return nc


_NC_CACHE = {}


def _get_nc(repeats: int = 1):
    if repeats not in _NC_CACHE:
        nc = bacc.Bacc("TRN2", target_bir_lowering=False, debug=False)
        _emit_kernel(nc, repeats)
        nc.compile()
        _NC_CACHE[repeats] = nc
    return _NC_CACHE[repeats]


def _rope_tables(pos):
    """pos: [N, 2] int -> cos/sin tables [128, N] f32, channel-major.

    Channel c in [0,64): j = c % 32 selects rotate-half slot, jj = j % 16
    the frequency; c < 32 uses pos[:, 0] (y), else pos[:, 1] (x). Sign of
    sin folded in: negative for j < 16. Rows 64:128 repeat 0:64 (2 heads
    per 128-partition tile).
    """
    invf = 1.0 / (100.0 ** (np.arange(0, 32, 2, dtype=np.float32) / 32.0))
    ay = (pos[:, 0].astype(np.float32)[None, :] * invf[:, None])  # [16, N]
    ax = (pos[:, 1].astype(np.float32)[None, :] * invf[:, None])
    cos = np.concatenate([np.cos(ay), np.cos(ay), np.cos(ax), np.cos(ax)], 0)
    sin = np.concatenate([-np.sin(ay), np.sin(ay), -np.sin(ax), np.sin(ax)], 0)
    cos = np.concatenate([cos, cos], 0).astype(np.float32)  # [128, N]
    sin = np.concatenate([sin, sin], 0).astype(np.float32)
    return np.ascontiguousarray(cos), np.ascontiguousarray(sin)


def _sw_x(xb, inner):
    # x[b].T is [C, N]; -> [N//inner, P, KIO, inner] so each chunk DMA
    # reads one 16/4KB contiguous run per partition
    xT = xb.T.reshape(KIO, P, xb.shape[0] // inner, inner)
    return np.ascontiguousarray(xT.transpose(2, 1, 0, 3))


def _sw_w(Wg):
    # W shard [CS, C]; lhsT layout [P, KIO, CS]
    return np.ascontiguousarray(Wg.T.reshape(KIO, P, CS).transpose(1, 0, 2))


def _build_in_maps(query, key, value, qpos, kpos, Wq, Wk, Wv, Wo):
    xqT = [_sw_x(query[b], NCHUNK) for b in range(B)]
    xkT = [_sw_x(key[b], NCHUNK) for b in range(B)]
    xvT = [_sw_x(value[b], P) for b in range(B)]
    tabs_q = [_rope_tables(np.asarray(qpos[b])) for b in range(B)]
    tabs_k = [_rope_tables(np.asarray(kpos[b])) for b in range(B)]
    wqT = [_sw_w(Wq[g * CS:(g + 1) * CS, :]) for g in range(G)]
    wkT = [_sw_w(Wk[g * CS:(g + 1) * CS, :]) for g in range(G)]
    wvT = [_sw_w(Wv[g * CS:(g + 1) * CS, :]) for g in range(G)]
    woT = [np.ascontiguousarray(
        Wo[:, g * CS:(g + 1) * CS].T.reshape(CS // P, P, C).transpose(1, 0, 2))
        for g in range(G)]

    in_maps = []
    for core in range(NCORES):
        b, g = core // G, core % G
        in_maps.append({
            "xq": xqT[b], "xk": xkT[b], "xv": xvT[b],
            "wq": wqT[g], "wk": wkT[g], "wv": wvT[g], "wo": woT[g],
            "cq": tabs_q[b][0], "sq": tabs_q[b][1],
            "ck": tabs_k[b][0], "sk": tabs_k[b][1],
        })
    return in_maps


def run_sharded(inputs, trace=False):
    """Run the SPMD kernel; returns (results, BassKernelResults)."""
    nc = _get_nc()
    in_maps = _build_in_maps(
        inputs["query"], inputs["key"], inputs["value"],
        inputs["qpos"], inputs["kpos"],
        inputs["Wq"], inputs["Wk"], inputs["Wv"], inputs["Wo"],
    )
    res = run_bass_kernel_spmd(nc, in_maps, list(range(NCORES)), trace=trace)
    return res


def _gather(results, bo):
    out = np.empty((B, NQ, C), np.float32)
    for b in range(B):
        out[b] = results[b * G]["out"] + results[b * G + 1]["out"] + bo
    return out


def kernel(query, key, value, qpos, kpos, Wq, Wk, Wv, Wo, bo):
    inputs = {
        "query": np.asarray(query, np.float32),
        "key": np.asarray(key, np.float32),
        "value": np.asarray(value, np.float32),
        "qpos": np.asarray(qpos),
        "kpos": np.asarray(kpos),
        "Wq": np.asarray(Wq, np.float32),
        "Wk": np.asarray(Wk, np.float32),
        "Wv": np.asarray(Wv, np.float32),
        "Wo": np.asarray(Wo, np.float32),
    }
    res = run_sharded(inputs, trace=False)
    return _gather(res.results, np.asarray(bo, np.float32))


# revision 7
# speedup vs baseline: 419.0552x; 1.5355x over previous
"""Trainium2 Bass kernel for nn_CrossAttention (CroCo RoPE2D cross-attention).

Sharding: 8 cores = batch(4) x head-group(2, 8 heads each).
Per core: QKV projections (fp32r matmuls), RoPE2D via host-precomputed
cos/sin tables (channel-major layout, stream_shuffle for rotate-half),
attention with scoresT [nk, nq] layout (row-packed K=64 head pairs),
exp on ACT with 1/8 scale folded in, PV + softmax sums via ones-matmul
(col-packed), normalization with fast reciprocal, output projection.

Output per core: partial out [2048, 1024] for its head group; host sums
the two group partials per batch and adds the bias.
"""

import numpy as np

import concourse.bass as bass
import concourse.mybir as mybir
import concourse.tile as tile
from concourse import bacc
from concourse.bass import ds, ts
from concourse.bass_utils import run_bass_kernel_spmd

B, NQ, NK, C = 4, 2048, 2048, 1024
H, DH = 16, 64
G = 2                      # head groups (tensor-parallel)
CS = C // G                # 512 channels per core
HPC = H // G               # 8 heads per core
NCORES = 8
P = 128
KIO = C // P               # 8 contraction subtiles for projections
NKT = NK // P              # 16 nk tiles
NCHUNK = 512               # nq chunk for attention stages
NCHUNKS = NQ // NCHUNK     # 4

f32 = mybir.dt.float32
f32r = mybir.dt.float32r
bf16 = mybir.dt.bfloat16
EXP = mybir.ActivationFunctionType.Exp
SWAP16 = [(i + 16) % 32 for i in range(32)]
SCALE = DH ** -0.5


def _emit_kernel(nc: bass.Bass, repeats: int = 1):
    xq = nc.dram_tensor("xq", [NCHUNKS, P, KIO, NCHUNK], f32, kind="ExternalInput").ap()
    xk = nc.dram_tensor("xk", [NCHUNKS, P, KIO, NCHUNK], f32, kind="ExternalInput").ap()
    xv = nc.dram_tensor("xv", [NKT, P, KIO, P], f32, kind="ExternalInput").ap()
    wq = nc.dram_tensor("wq", [P, KIO, CS], f32, kind="ExternalInput").ap()
    wk = nc.dram_tensor("wk", [P, KIO, CS], f32, kind="ExternalInput").ap()
    wv = nc.dram_tensor("wv", [P, KIO, CS], f32, kind="ExternalInput").ap()
    wo = nc.dram_tensor("wo", [P, CS // P, C], bf16, kind="ExternalInput").ap()
    cq = nc.dram_tensor("cq", [P, NQ], f32, kind="ExternalInput").ap()
    sq = nc.dram_tensor("sq", [P, NQ], f32, kind="ExternalInput").ap()
    ck = nc.dram_tensor("ck", [P, NK], f32, kind="ExternalInput").ap()
    sk = nc.dram_tensor("sk", [P, NK], f32, kind="ExternalInput").ap()
    out = nc.dram_tensor("out", [NQ, C], f32, kind="ExternalOutput").ap()

    xq_r, xk_r, xv_r = xq, xk, xv
    wq_r = wq.bitcast(f32r)
    wk_r = wk.bitcast(f32r)
    wv_r = wv.bitcast(f32r)
    wo_r = wo

    with tile.TileContext(nc) as tc:
      for _rep in range(repeats):
        px = f"r{_rep}_" if repeats > 1 else ""
        with tc.tile_pool(name=px + "persist", bufs=1) as persist, \
             tc.tile_pool(name=px + "stream", bufs=2) as stream, \
             tc.tile_pool(name=px + "ropep", bufs=2) as ropep, \
             tc.tile_pool(name=px + "tabs", bufs=1) as tabs, \
             tc.tile_pool(name=px + "sbB", bufs=1) as sbB, \
             tc.tile_pool(name=px + "psA", bufs=2, space="PSUM") as psA, \
             tc.tile_pool(name=px + "psB", bufs=1, space="PSUM") as psB:
            qT = [persist.tile([P, NQ], f32r, name=f"{px}qT{i}", tag=f"{px}qT{i}")
                  for i in range(4)]
            kT = [persist.tile([P, NK], f32r, name=f"{px}kT{i}", tag=f"{px}kT{i}")
                  for i in range(4)]
            v_sb = persist.tile([P, NKT, CS], bf16, name=px + "v_sb", tag=px + "v_sb")
            ones = persist.tile([P, 64], bf16, name=px + "ones", tag=px + "ones")
            nc.vector.memset(ones[:], 1.0)

            xn = [[None] * 4 for _ in range(NCHUNKS)]  # [ch][pair]

            def proj_qk(x_r, w_r, cos_dram, sin_dram, dstT, wtag, cts):
                cos_sb = tabs.tile([P, NQ], f32, name=f"{px}cos_{wtag}",
                                   tag=px + "cos", bufs=1)
                sin_sb = tabs.tile([P, NQ], f32, name=f"{px}sin_{wtag}",
                                   tag=px + "sin", bufs=1)
                nc.sync.dma_start(cos_sb[:], cos_dram)
                nc.sync.dma_start(sin_sb[:], sin_dram)
                w_sb = stream.tile([P, KIO, CS], f32r, name=f"{px}w_{wtag}{cts[0]}",
                                   tag=px + "w", bufs=1)
                nc.sync.dma_start(w_sb[:], w_r)
                for ch in range(NCHUNKS):
                    x_sb = stream.tile([P, KIO, NCHUNK], f32r,
                                       name=f"{px}x_{wtag}{cts[0]}{ch}", tag=px + "x",
                                       bufs=2)
                    nc.sync.dma_start(x_sb[:], x_r[ch].bitcast(f32r))
                    for ct in cts:
                        pst = psA.tile([P, NCHUNK], f32, name=f"{px}pj{wtag}{ch}{ct}",
                                       tag=px + "proj", bufs=2)
                        for kio in range(KIO):
                            nc.tensor.matmul(
                                pst[:], w_sb[:, kio, ds(ct * P, P)], x_sb[:, kio, :],
                                start=(kio == 0), stop=(kio == KIO - 1),
                            )
                        qs = ropep.tile([P, NCHUNK], f32, name=f"{px}qs{wtag}{ch}{ct}",
                                        tag=px + "qs", bufs=2)
                        nc.vector.stream_shuffle(qs[:], pst[:], SWAP16)
                        t1 = ropep.tile([P, NCHUNK], f32, name=f"{px}t1{wtag}{ch}{ct}",
                                        tag=px + "t1", bufs=2)
                        nc.vector.tensor_mul(
                            out=t1[:], in0=pst[:],
                            in1=cos_sb[:, ds(ch * NCHUNK, NCHUNK)],
                        )
                        t2 = ropep.tile([P, NCHUNK], f32, name=f"{px}t2{wtag}{ch}{ct}",
                                        tag=px + "t2", bufs=2)
                        nc.vector.tensor_mul(
                            out=t2[:], in0=qs[:],
                            in1=sin_sb[:, ds(ch * NCHUNK, NCHUNK)],
                        )
                        nc.vector.tensor_add(
                            out=dstT[ct][:, ds(ch * NCHUNK, NCHUNK)],
                            in0=t1[:], in1=t2[:],
                        )

            def proj_v():
                wv_sb = stream.tile([P, KIO, CS], f32r, name=px + "wv_sb",
                                    tag=px + "w", bufs=1)
                nc.sync.dma_start(wv_sb[:], wv_r)
                for t in range(NKT):
                    xv_sb = stream.tile([P, KIO, P], f32r, name=f"{px}xv{t}",
                                        tag=px + "xv", bufs=2)
                    nc.sync.dma_start(xv_sb[:], xv_r[t].bitcast(f32r))
                    psv = psA.tile([P, CS], f32, name=f"{px}psv{t}", tag=px + "proj",
                                   bufs=2)
                    for kio in range(KIO):
                        nc.tensor.matmul(
                            psv[:], xv_sb[:, kio, :], wv_sb[:, kio, :],
                            start=(kio == 0), stop=(kio == KIO - 1),
                        )
                    nc.vector.tensor_copy(v_sb[:, t, :], psv[:])

            def wo_chunk(ch, woT):
                for ns in range(NCHUNK // P):
                    o_sb = sbB.tile([P, C], f32, name=f"{px}o{ch}{ns}",
                                    tag=px + "osb", bufs=1)
                    for oc in range(2):
                        wops = psB.tile([P, 512], f32, name=f"{px}wops{ch}{ns}{oc}",
                                        tag=px + "sums", bufs=1)
                        for ct in range(4):
                            nc.tensor.matmul(
                                wops[:], xn[ch][ct][:, ds(ns * P, P)],
                                woT[:, ct, ds(oc * 512, 512)],
                                start=(ct == 0), stop=(ct == 3),
                            )
                        nc.vector.tensor_copy(o_sb[:, ds(oc * 512, 512)], wops[:])
                    nc.sync.dma_start(out[ds(ch * NCHUNK + ns * P, P), :], o_sb[:])

            def attention_pair(pair, woT=None):
                for ch in range(NCHUNKS):
                    nq_sl = ds(ch * NCHUNK, NCHUNK)
                    xn[ch][pair] = sbB.tile([P, NCHUNK], bf16,
                                            name=f"{px}xn{ch}_{pair}",
                                            tag=f"{px}xn{ch}_{pair}", bufs=1)
                    pv = psB.tile([P, NCHUNK], f32, name=f"{px}pv{ch}{pair}",
                                  tag=px + "pv", bufs=1)
                    sums = psB.tile([P, NCHUNK], f32, name=f"{px}sm{ch}{pair}",
                                    tag=px + "sums", bufs=1)
                    for t in range(NKT):
                        qk = psB.tile([P, 2 * NCHUNK], f32,
                                      name=f"{px}qk{ch}{pair}{t}", tag=px + "qk",
                                      bufs=2)
                        nc.tensor.matmul(
                            qk[:, 0:NCHUNK], kT[pair][0:64, ds(t * P, P)],
                            qT[pair][0:64, nq_sl], start=True, stop=True,
                        )
                        nc.tensor.matmul(
                            qk[:, NCHUNK:2 * NCHUNK], kT[pair][64:128, ds(t * P, P)],
                            qT[pair][64:128, nq_sl], start=True, stop=True,
                        )
                        e = sbB.tile([P, 2 * NCHUNK], bf16, name=f"{px}e{ch}{pair}{t}",
                                     tag=px + "e", bufs=3)
                        nc.scalar.activation(e[:], qk[:], EXP, scale=SCALE)
                        hA, hB = 2 * pair, 2 * pair + 1
                        nc.tensor.matmul(
                            pv[0:64, :], v_sb[:, t, ds(hA * DH, DH)], e[:, 0:NCHUNK],
                            start=(t == 0), stop=(t == NKT - 1),
                        )
                        nc.tensor.matmul(
                            pv[64:128, :], v_sb[:, t, ds(hB * DH, DH)],
                            e[:, NCHUNK:2 * NCHUNK],
                            start=(t == 0), stop=(t == NKT - 1),
                        )
                        nc.tensor.matmul(
                            sums[0:64, :], ones[:], e[:, 0:NCHUNK],
                            start=(t == 0), stop=(t == NKT - 1),
                        )
                        nc.tensor.matmul(
                            sums[64:128, :], ones[:], e[:, NCHUNK:2 * NCHUNK],
                            start=(t == 0), stop=(t == NKT - 1),
                        )
                    recips = sbB.tile([P, NCHUNK], f32, name=f"{px}rc{ch}{pair}",
                                      tag=px + "recips", bufs=2)
                    nc.vector.reciprocal_approx_fast(out=recips[:], in_=sums[:])
                    nc.vector.tensor_mul(
                        out=xn[ch][pair][:], in0=pv[:], in1=recips[:],
                    )
                    if woT is not None:
                        wo_chunk(ch, woT)

            # --- emission: pair-0 inputs first, attention overlaps the rest ---
            proj_qk(xq_r, wq_r, cq, sq, qT, "q", [0])
            proj_qk(xk_r, wk_r, ck, sk, kT, "k", [0])
            proj_v()
            attention_pair(0)
            proj_qk(xq_r, wq_r, cq, sq, qT, "q", [1, 2, 3])
            proj_qk(xk_r, wk_r, ck, sk, kT, "k", [1, 2, 3])
            attention_pair(1)
            attention_pair(2)
            woT = sbB.tile([P, CS // P, C], bf16, name=px + "woT", tag=px + "woT",
                           bufs=1)
            nc.sync.dma_start(woT[:], wo_r)
            attention_pair(3, woT=woT)
    return nc


_NC_CACHE = {}


def _get_nc(repeats: int = 1):
    if repeats not in _NC_CACHE:
        nc = bacc.Bacc("TRN2", target_bir_lowering=False, debug=False)
        _emit_kernel(nc, repeats)
        nc.compile()
        _NC_CACHE[repeats] = nc
    return _NC_CACHE[repeats]


def _rope_tables(pos):
    """pos: [N, 2] int -> cos/sin tables [128, N] f32, channel-major.

    Channel c in [0,64): j = c % 32 selects rotate-half slot, jj = j % 16
    the frequency; c < 32 uses pos[:, 0] (y), else pos[:, 1] (x). Sign of
    sin folded in: negative for j < 16. Rows 64:128 repeat 0:64 (2 heads
    per 128-partition tile).
    """
    invf = 1.0 / (100.0 ** (np.arange(0, 32, 2, dtype=np.float32) / 32.0))
    ay = (pos[:, 0].astype(np.float32)[None, :] * invf[:, None])  # [16, N]
    ax = (pos[:, 1].astype(np.float32)[None, :] * invf[:, None])
    cos = np.concatenate([np.cos(ay), np.cos(ay), np.cos(ax), np.cos(ax)], 0)
    sin = np.concatenate([-np.sin(ay), np.sin(ay), -np.sin(ax), np.sin(ax)], 0)
    cos = np.concatenate([cos, cos], 0).astype(np.float32)  # [128, N]
    sin = np.concatenate([sin, sin], 0).astype(np.float32)
    return np.ascontiguousarray(cos), np.ascontiguousarray(sin)


def _sw_x(xb, inner):
    # x[b].T is [C, N]; -> [N//inner, P, KIO, inner] so each chunk DMA
    # reads one 16/4KB contiguous run per partition
    xT = xb.T.reshape(KIO, P, xb.shape[0] // inner, inner)
    return np.ascontiguousarray(xT.transpose(2, 1, 0, 3))


def _sw_w(Wg):
    # W shard [CS, C]; lhsT layout [P, KIO, CS]
    return np.ascontiguousarray(Wg.T.reshape(KIO, P, CS).transpose(1, 0, 2))


def _build_in_maps(query, key, value, qpos, kpos, Wq, Wk, Wv, Wo):
    xqT = [_sw_x(query[b], NCHUNK) for b in range(B)]
    xkT = [_sw_x(key[b], NCHUNK) for b in range(B)]
    xvT = [_sw_x(value[b], P) for b in range(B)]
    tabs_q = [_rope_tables(np.asarray(qpos[b])) for b in range(B)]
    tabs_k = [_rope_tables(np.asarray(kpos[b])) for b in range(B)]
    wqT = [_sw_w(Wq[g * CS:(g + 1) * CS, :]) for g in range(G)]
    wkT = [_sw_w(Wk[g * CS:(g + 1) * CS, :]) for g in range(G)]
    wvT = [_sw_w(Wv[g * CS:(g + 1) * CS, :]) for g in range(G)]
    import ml_dtypes
    woT = [np.ascontiguousarray(
        Wo[:, g * CS:(g + 1) * CS].T.reshape(CS // P, P, C).transpose(1, 0, 2)
        .astype(ml_dtypes.bfloat16)) for g in range(G)]

    in_maps = []
    for core in range(NCORES):
        b, g = core // G, core % G
        in_maps.append({
            "xq": xqT[b], "xk": xkT[b], "xv": xvT[b],
            "wq": wqT[g], "wk": wkT[g], "wv": wvT[g], "wo": woT[g],
            "cq": tabs_q[b][0], "sq": tabs_q[b][1],
            "ck": tabs_k[b][0], "sk": tabs_k[b][1],
        })
    return in_maps


def run_sharded(inputs, trace=False):
    """Run the SPMD kernel; returns (results, BassKernelResults)."""
    nc = _get_nc()
    in_maps = _build_in_maps(
        inputs["query"], inputs["key"], inputs["value"],
        inputs["qpos"], inputs["kpos"],
        inputs["Wq"], inputs["Wk"], inputs["Wv"], inputs["Wo"],
    )
    res = run_bass_kernel_spmd(nc, in_maps, list(range(NCORES)), trace=trace)
    return res


def _gather(results, bo):
    out = np.empty((B, NQ, C), np.float32)
    for b in range(B):
        out[b] = results[b * G]["out"] + results[b * G + 1]["out"] + bo
    return out


def kernel(query, key, value, qpos, kpos, Wq, Wk, Wv, Wo, bo):
    inputs = {
        "query": np.asarray(query, np.float32),
        "key": np.asarray(key, np.float32),
        "value": np.asarray(value, np.float32),
        "qpos": np.asarray(qpos),
        "kpos": np.asarray(kpos),
        "Wq": np.asarray(Wq, np.float32),
        "Wk": np.asarray(Wk, np.float32),
        "Wv": np.asarray(Wv, np.float32),
        "Wo": np.asarray(Wo, np.float32),
    }
    res = run_sharded(inputs, trace=False)
    return _gather(res.results, np.asarray(bo, np.float32))
